# revision 2
# baseline (speedup 1.0000x reference)
"""Trainium2 Bass kernel for nn_AttentionModel (S=2048, B=32, H=1024).

Math: reference computes
    energy[b,s] = (enc[s,b,:] @ We.T + (h @ Wh.T + bias)) @ v  ; out = softmax_s(energy)
Since softmax is shift-invariant and the (h @ Wh.T + bias) @ v term is constant
over s, the output reduces exactly to
    out[b, 0, s] = softmax_s( enc[s,b,:] . u ),   u = v[0] @ We   (We = attn_W[:, H:])
So the kernel is a memory-bound [S*B, H] x [H] matvec + row softmax.

Sharding: data-parallel over batch B across 8 cores (4 batches/core).

Device-side design (per core):
- enc streamed in fp16 (host casts; softmax tolerance has ~8x margin) --
  halves the HBM traffic, which is the roofline for this kernel.
- The stream is 16 uniform 1MB DMAs, one per (batch, 512-col slice).
  Host lays enc out as [BL, 128, ns, jc, 512]: partition p holds rows
  h = j*128+p for one slice contiguously, so each DMA is 128 descriptors
  of jc*512*2 = 8KB contiguous bytes -- near-peak HBM rate -- and the
  dependency granularity is one slice, so the post-stream tail is just
  one slice's matmuls + add + exp + store. A deep tile pool keeps the
  DMA queue full regardless of PE hiccups.
- PE column tiling 2x: h-chunks 0..3 accumulate on PSUM partition 64 (PE
  col group 64), chunks 4..7 on partition 0 (col group 0). The two groups'
  matmuls run concurrently in disjoint PE column groups, halving PE time so
  the kernel stays DMA-bound.
- PSUM is allocated per (batch, 512-slice) -- one bank each, 8 banks in
  flight -- so PSUM recycles slice-by-slice and the epilogue never gates
  the next slice's matmuls.
- Epilogue per slice: when group 64 finishes (chunk 3), DVE stages its
  partial to SBUF; when group 0 finishes (chunk 7), DVE adds the partials
  and ACT computes exp(e - 44) (constant bias -- the energies stay inside
  exp's f32 range, and the constant cancels in the host normalization).
  The host sums the returned exp values for the softmax denominator.
- Mid-stream outputs ride the idle SWDGE (gpsimd) ring; the last batch
  ships per-slice on the sync ring, which is idle once the enc stream ends.
"""

import numpy as np

import concourse.bass as bass
import concourse.tile as tile
from concourse import bacc, mybir
from concourse.bass_utils import run_bass_kernel_spmd

S, B, H = 2048, 32, 1024
NCORES = 8
BL = B // NCORES  # batches per core
MM_N = 512        # matmul moving free dim (one fp32 PSUM bank)
EXP_BIAS = -44.0  # constant shift inside exp; cancels in host normalization


def build_nc(bl=BL, h=H, s=S, enc_bufs=12):
    """Build the per-core Bass program (SPMD: same program, different data)."""
    nc = bacc.Bacc()
    f32 = mybir.dt.float32
    f16 = mybir.dt.float16
    jc = h // 128      # h chunks (contraction tiles)
    ns = s // MM_N     # 512-wide slices per output row

    enc_d = nc.declare_dram_parameter("enc", [bl, 128, ns, jc, MM_N], f16,
                                      isOutput=False)
    u_d = nc.declare_dram_parameter("u", [128, jc], f16, isOutput=False)
    out_d = nc.declare_dram_parameter("out", [bl, s], f32, isOutput=True)

    with tile.TileContext(nc) as tc:
        with (
            tc.tile_pool(name="up", bufs=1) as up,
            tc.tile_pool(name="encp", bufs=enc_bufs) as encp,
            tc.tile_pool(name="smp", bufs=2) as smp,
            tc.tile_pool(name="psp", bufs=8, space="PSUM") as psp,
        ):
            # First enc load goes out immediately on the sync ring; the tiny
            # u load rides the second HWDGE ring (ACT) in parallel.
            t0 = encp.tile([128, jc, MM_N], f16, name="t")
            nc.sync.dma_start(t0[:], enc_d[0, :, 0, :, :])
            u_sb = up.tile([128, jc], f16)
            nc.scalar.dma_start(u_sb[:], u_d[:])
            bias_sb = up.tile([1, 1], f32)
            nc.gpsimd.memset(bias_sb[:], EXP_BIAS)

            for b in range(bl):
                t64 = smp.tile([1, s], f32)
                p_exp = smp.tile([1, s], f32)
                for ss in range(ns):
                    if b == 0 and ss == 0:
                        t = t0
                    else:
                        t = encp.tile([128, jc, MM_N], f16, name="t")
                        nc.sync.dma_start(t[:], enc_d[b, :, ss, :, :])
                    e_ps = psp.tile([128, MM_N], f32, name="eps")
                    sl = slice(ss * MM_N, (ss + 1) * MM_N)
                    for j in range(jc):
                        gp = 64 if j < jc // 2 else 0
                        nc.tensor.matmul(
                            e_ps[gp:gp + 1, :],
                            u_sb[:, j:j + 1],
                            t[:, j, :],
                            start=j in (0, jc // 2),
                            stop=j in (jc // 2 - 1, jc - 1),
                            tile_position=(0, gp),
                        )
                        if j == jc // 2 - 1:
                            # Group 64 done for this slice: stage its partial
                            # in SBUF while group 0 streams. (TensorTensor can
                            # read only one PSUM input, so the add below needs
                            # this.)
                            nc.vector.tensor_copy(t64[:, sl], e_ps[64:65, :])
                        if j == jc - 1:
                            # Slice complete: merge col-group partials,
                            # exp(e - 44), ship.
                            nc.vector.tensor_tensor(
                                e_ps[32:33, :],
                                e_ps[0:1, :], t64[:, sl],
                                op=mybir.AluOpType.add,
                            )
                            nc.scalar.activation(
                                p_exp[:, sl], e_ps[32:33, :],
                                mybir.ActivationFunctionType.Exp,
                                bias=bias_sb[:],
                            )
                            if b == bl - 1:
                                # Tail batch: ship each slice as soon as its
                                # exp lands. These ride the sync ring -- idle
                                # once the enc stream ends -- so the DMA
                                # issues don't serialize between exps on the
                                # ACT queue.
                                nc.sync.dma_start(
                                    out_d[b:b + 1, sl], p_exp[:, sl])
                if b != bl - 1:
                    # Mid-stream outputs ride the idle SWDGE (gpsimd) ring so
                    # their issue slots never sit between exps on the ACT
                    # queue nor behind enc loads on the sync ring.
                    nc.gpsimd.dma_start(out_d[b:b + 1, :], p_exp[:])
    nc.compile()
    return nc


def _prep_inputs(encoder_outputs, attn_W, v):
    encoder_outputs = np.asarray(encoder_outputs, dtype=np.float32)
    attn_W = np.asarray(attn_W, dtype=np.float32)
    v = np.asarray(v, dtype=np.float32)
    h = attn_W.shape[0]
    jc = h // 128
    ns = S // MM_N
    # u = v[0] @ We in float64 (host-side, tiny)
    u = (v[0].astype(np.float64) @ attn_W[:, h:].astype(np.float64)).astype(np.float16)
    u128 = np.ascontiguousarray(u.reshape(jc, 128).T)  # [128, jc]
    in_maps = []
    for c in range(NCORES):
        sl = encoder_outputs[:, c * BL:(c + 1) * BL, :]
        enc_c = sl.transpose(1, 2, 0).astype(np.float16)   # [BL, H, S]
        # [BL, H, S] -> [BL, 128, ns, jc, 512]: partition p holds rows
        # h = j*128+p of slice ss as one contiguous 8KB run per DMA.
        enc_c = np.ascontiguousarray(
            enc_c.reshape(BL, jc, 128, ns, MM_N).transpose(0, 2, 3, 1, 4))
        in_maps.append({"enc": enc_c, "u": u128})
    return in_maps


def run(encoder_outputs, rnn_hidden, attn_W, attn_b, v, trace=False, **bass_kwargs):
    in_maps = _prep_inputs(encoder_outputs, attn_W, v)
    nc = build_nc()
    res = run_bass_kernel_spmd(
        nc, in_maps, list(range(NCORES)), trace=trace, **bass_kwargs
    )
    num = np.concatenate([r["out"] for r in res.results], axis=0)    # [B, S]
    # normalize on host: the constant exp bias cancels in the division
    num = num.astype(np.float64)
    out = num / num.sum(axis=1, keepdims=True)
    return out[:, None, :].astype(np.float32), res


def kernel(encoder_outputs, rnn_hidden, attn_W, attn_b, v):
    out, _ = run(encoder_outputs, rnn_hidden, attn_W, attn_b, v)
    return out


# revision 3
# speedup vs baseline: 1.0844x; 1.0844x over previous
"""Trainium2 Bass kernel for nn_AttentionModel (S=2048, B=32, H=1024).

Math: reference computes
    energy[b,s] = (enc[s,b,:] @ We.T + (h @ Wh.T + bias)) @ v  ; out = softmax_s(energy)
Since softmax is shift-invariant and the (h @ Wh.T + bias) @ v term is constant
over s, the output reduces exactly to
    out[b, 0, s] = softmax_s( enc[s,b,:] . u ),   u = v[0] @ We   (We = attn_W[:, H:])
So the kernel is a memory-bound [S*B, H] x [H] matvec + row softmax.

Sharding: data-parallel over batch B across 8 cores (4 batches/core).

Device-side design (per core):
- enc streamed in fp16 (host casts; softmax tolerance has ~8x margin) --
  halves the HBM traffic, which is the roofline for this kernel.
- The stream is 16 uniform 1MB DMAs, one per (batch, 512-col slice).
  Host lays enc out as [BL, 128, ns, jc, 512]: partition p holds rows
  h = j*128+p for one slice contiguously, so each DMA is 128 descriptors
  of jc*512*2 = 8KB contiguous bytes -- near-peak HBM rate -- and the
  dependency granularity is one slice, so the post-stream tail is just
  one slice-pair's matmuls + exp + store. A deep tile pool keeps the
  DMA queue full regardless of PE hiccups.
- PE column tiling 2x BY SLICE: even slices accumulate their full
  8-chunk contraction on PE col group 64 (PSUM partition 64), odd slices
  on group 0. The two groups' matmuls run concurrently in disjoint PE
  column groups, halving PE time, and -- unlike a by-chunk split -- no
  cross-group partial merge is needed: the epilogue is a single exp per
  slice, so Vector never sits on the critical path.
- PSUM is allocated per slice -- one bank each, 8 banks in flight -- so
  PSUM recycles slice-by-slice with ~8 slices of pipeline slack.
- Epilogue per slice: ACT computes exp(e - 44) straight out of PSUM
  (constant bias -- the energies stay inside exp's f32 range, and the
  constant cancels in the host normalization). The host sums the
  returned exp values for the softmax denominator.
- Mid-stream outputs ride the idle SWDGE (gpsimd) ring; the last batch
  ships per-slice on the sync ring, which is idle once the enc stream
  ends.
"""

import numpy as np

import concourse.bass as bass
import concourse.tile as tile
from concourse import bacc, mybir
from concourse.bass_utils import run_bass_kernel_spmd

S, B, H = 2048, 32, 1024
NCORES = 8
BL = B // NCORES  # batches per core
MM_N = 512        # matmul moving free dim (one fp32 PSUM bank)
EXP_BIAS = -44.0  # constant shift inside exp; cancels in host normalization


def build_nc(bl=BL, h=H, s=S, enc_bufs=14):
    """Build the per-core Bass program (SPMD: same program, different data)."""
    nc = bacc.Bacc()
    f32 = mybir.dt.float32
    f16 = mybir.dt.float16
    jc = h // 128      # h chunks (contraction tiles)
    ns = s // MM_N     # 512-wide slices per output row

    enc_d = nc.declare_dram_parameter("enc", [bl, 128, ns, jc, MM_N], f16,
                                      isOutput=False)
    u_d = nc.declare_dram_parameter("u", [128, jc], f16, isOutput=False)
    out_d = nc.declare_dram_parameter("out", [bl, s], f32, isOutput=True)

    with tile.TileContext(nc) as tc:
        with (
            tc.tile_pool(name="up", bufs=1) as up,
            tc.tile_pool(name="encp", bufs=enc_bufs) as encp,
            tc.tile_pool(name="smp", bufs=2) as smp,
            tc.tile_pool(name="psp", bufs=8, space="PSUM") as psp,
        ):
            # First enc load goes out immediately on the sync ring; the tiny
            # u load rides the second HWDGE ring (ACT) in parallel.
            t0 = encp.tile([128, jc, MM_N], f16, name="t")
            nc.sync.dma_start(t0[:], enc_d[0, :, 0, :, :])
            u_sb = up.tile([128, jc], f16)
            nc.scalar.dma_start(u_sb[:], u_d[:])
            bias_sb = up.tile([1, 1], f32)
            nc.gpsimd.memset(bias_sb[:], EXP_BIAS)

            for b in range(bl):
                p_exp = smp.tile([1, s], f32)
                for pair in range(ns // 2):
                    sA, sB = 2 * pair, 2 * pair + 1
                    if b == 0 and pair == 0:
                        tA = t0
                    else:
                        tA = encp.tile([128, jc, MM_N], f16, name="t")
                        nc.sync.dma_start(tA[:], enc_d[b, :, sA, :, :])
                    tB = encp.tile([128, jc, MM_N], f16, name="t")
                    nc.sync.dma_start(tB[:], enc_d[b, :, sB, :, :])
                    eA = psp.tile([128, MM_N], f32, name="eps")
                    eB = psp.tile([128, MM_N], f32, name="eps")
                    # Slice A accumulates on PE col group 64, slice B on
                    # group 0; the j-interleave keeps both groups streaming
                    # concurrently.
                    for j in range(jc):
                        nc.tensor.matmul(
                            eA[64:65, :], u_sb[:, j:j + 1], tA[:, j, :],
                            start=j == 0, stop=j == jc - 1,
                            tile_position=(0, 64),
                        )
                        nc.tensor.matmul(
                            eB[0:1, :], u_sb[:, j:j + 1], tB[:, j, :],
                            start=j == 0, stop=j == jc - 1,
                            tile_position=(0, 0),
                        )
                    for ss, eps, gp in ((sA, eA, 64), (sB, eB, 0)):
                        sl = slice(ss * MM_N, (ss + 1) * MM_N)
                        nc.scalar.activation(
                            p_exp[:, sl], eps[gp:gp + 1, :],
                            mybir.ActivationFunctionType.Exp,
                            bias=bias_sb[:],
                        )
                        if b == bl - 1:
                            # Tail batch: ship each slice as soon as its exp
                            # lands. These ride the sync ring -- idle once
                            # the enc stream ends -- so the DMA issues don't
                            # serialize between exps on the ACT queue.
                            nc.sync.dma_start(out_d[b:b + 1, sl],
                                              p_exp[:, sl])
                if b != bl - 1:
                    # Mid-stream outputs ride the idle SWDGE (gpsimd) ring so
                    # their issue slots never sit between exps on the ACT
                    # queue nor behind enc loads on the sync ring.
                    nc.gpsimd.dma_start(out_d[b:b + 1, :], p_exp[:])
    nc.compile()
    return nc


def _prep_inputs(encoder_outputs, attn_W, v):
    encoder_outputs = np.asarray(encoder_outputs, dtype=np.float32)
    attn_W = np.asarray(attn_W, dtype=np.float32)
    v = np.asarray(v, dtype=np.float32)
    h = attn_W.shape[0]
    jc = h // 128
    ns = S // MM_N
    # u = v[0] @ We in float64 (host-side, tiny)
    u = (v[0].astype(np.float64) @ attn_W[:, h:].astype(np.float64)).astype(np.float16)
    u128 = np.ascontiguousarray(u.reshape(jc, 128).T)  # [128, jc]
    in_maps = []
    for c in range(NCORES):
        sl = encoder_outputs[:, c * BL:(c + 1) * BL, :]
        enc_c = sl.transpose(1, 2, 0).astype(np.float16)   # [BL, H, S]
        # [BL, H, S] -> [BL, 128, ns, jc, 512]: partition p holds rows
        # h = j*128+p of slice ss as one contiguous 8KB run per DMA.
        enc_c = np.ascontiguousarray(
            enc_c.reshape(BL, jc, 128, ns, MM_N).transpose(0, 2, 3, 1, 4))
        in_maps.append({"enc": enc_c, "u": u128})
    return in_maps


def run(encoder_outputs, rnn_hidden, attn_W, attn_b, v, trace=False, **bass_kwargs):
    in_maps = _prep_inputs(encoder_outputs, attn_W, v)
    nc = build_nc()
    res = run_bass_kernel_spmd(
        nc, in_maps, list(range(NCORES)), trace=trace, **bass_kwargs
    )
    num = np.concatenate([r["out"] for r in res.results], axis=0)    # [B, S]
    # normalize on host: the constant exp bias cancels in the division
    num = num.astype(np.float64)
    out = num / num.sum(axis=1, keepdims=True)
    return out[:, None, :].astype(np.float32), res


def kernel(encoder_outputs, rnn_hidden, attn_W, attn_b, v):
    out, _ = run(encoder_outputs, rnn_hidden, attn_W, attn_b, v)
    return out


# revision 8
# speedup vs baseline: 1.1711x; 1.0800x over previous
"""Trainium2 Bass kernel for nn_AttentionModel (S=2048, B=32, H=1024).

Math: reference computes
    energy[b,s] = (enc[s,b,:] @ We.T + (h @ Wh.T + bias)) @ v  ; out = softmax_s(energy)
Since softmax is shift-invariant and the (h @ Wh.T + bias) @ v term is constant
over s, the output reduces exactly to
    out[b, 0, s] = softmax_s( enc[s,b,:] . u ),   u = v[0] @ We   (We = attn_W[:, H:])
So the kernel is a memory-bound [S*B, H] x [H] matvec + row softmax.

Sharding: data-parallel over batch B across 8 cores (4 batches/core).

Device-side design (per core):
- enc streamed in fp16 (host casts; softmax tolerance has ~8x margin) --
  halves the HBM traffic, which is the roofline for this kernel.
- The stream is 32 uniform 1MB DMAs, two per (batch, 1024-col slice
  pair). Host lays enc out as [BL, 128, np, jc, 1024]: partition p holds
  rows h = j*128+p for one slice pair contiguously, so each DMA is 128
  descriptors of 8KB contiguous bytes -- near-peak HBM rate -- and the
  dependency granularity is half a pair, so the post-stream tail is just
  4 chunk's matmuls + exp + store. A deep tile pool keeps the DMA queue
  full regardless of PE hiccups.
- PE column tiling 2x BY SLICE: the pair's even slice accumulates its
  full 8-chunk contraction on PE col group 64 (PSUM partition 64), the
  odd slice on group 0. Both groups' matmuls depend on the same DMAs, so
  the Tile scheduler interleaves them and they run concurrently in
  disjoint PE column groups, halving PE time; unlike a by-chunk split,
  no cross-group partial merge is needed: the epilogue is a single exp
  per slice, so Vector never sits on the critical path.
- PSUM is allocated per slice -- one bank each, 8 banks in flight -- so
  PSUM recycles slice-by-slice with ~8 slices of pipeline slack.
- Epilogue per slice: ACT computes exp(e - 44) straight out of PSUM
  (constant bias -- the energies stay inside exp's f32 range, and the
  constant cancels in the host normalization). The host sums the
  returned exp values for the softmax denominator.
- Mid-stream outputs ride the idle SWDGE (gpsimd) ring; the last batch
  ships per-slice on the sync ring, which is idle once the enc stream
  ends.
"""

import numpy as np

import concourse.bass as bass
import concourse.tile as tile
from concourse import bacc, mybir
from concourse.bass_utils import run_bass_kernel_spmd

S, B, H = 2048, 32, 1024
NCORES = 8
BL = B // NCORES  # batches per core
MM_N = 512        # matmul moving free dim (one fp32 PSUM bank)
EXP_BIAS = -44.0  # constant shift inside exp; cancels in host normalization


def build_nc(bl=BL, h=H, s=S, enc_bufs=8):
    """Build the per-core Bass program (SPMD: same program, different data)."""
    nc = bacc.Bacc()
    f32 = mybir.dt.float32
    f16 = mybir.dt.float16
    jc = h // 128      # h chunks (contraction tiles)
    ns = s // MM_N     # 512-wide slices per output row
    np_ = ns // 2      # slice pairs per output row

    enc_d = nc.declare_dram_parameter("enc", [bl, 128, np_, jc, 2 * MM_N],
                                      f16, isOutput=False)
    u_d = nc.declare_dram_parameter("u", [128, jc], f16, isOutput=False)
    out_d = nc.declare_dram_parameter("out", [bl, s], f32, isOutput=True)

    with tile.TileContext(nc) as tc:
        with (
            tc.tile_pool(name="up", bufs=1) as up,
            tc.tile_pool(name="encp", bufs=enc_bufs) as encp,
            tc.tile_pool(name="smp", bufs=2) as smp,
            tc.tile_pool(name="psp", bufs=8, space="PSUM") as psp,
        ):
            # First enc load goes out immediately on the sync ring; the tiny
            # u load rides the second HWDGE ring (ACT) in parallel.
            jh = jc // 2
            t0 = encp.tile([128, jc, 2 * MM_N], f16, name="t")
            nc.sync.dma_start(t0[:, 0:jh, :], enc_d[0, :, 0, 0:jh, :])
            u_sb = up.tile([128, jc], f16)
            nc.scalar.dma_start(u_sb[:], u_d[:])
            bias_sb = up.tile([1, 1], f32)
            nc.gpsimd.memset(bias_sb[:], EXP_BIAS)
            nc.sync.dma_start(t0[:, jh:jc, :], enc_d[0, :, 0, jh:jc, :])

            for b in range(bl):
                p_exp = smp.tile([1, s], f32)
                for pair in range(np_):
                    sA, sB = 2 * pair, 2 * pair + 1
                    if b == 0 and pair == 0:
                        t = t0
                    else:
                        # Two 1MB half-DMAs per pair tile: finer dependency
                        # granularity so the first 4 chunks' matmuls start
                        # while the rest stream.
                        t = encp.tile([128, jc, 2 * MM_N], f16, name="t")
                        nc.sync.dma_start(t[:, 0:jh, :],
                                          enc_d[b, :, pair, 0:jh, :])
                        nc.sync.dma_start(t[:, jh:jc, :],
                                          enc_d[b, :, pair, jh:jc, :])
                    eA = psp.tile([128, MM_N], f32, name="eps")
                    eB = psp.tile([128, MM_N], f32, name="eps")
                    # Slice A accumulates on PE col group 64, slice B on
                    # group 0; the j-interleave keeps both groups streaming
                    # concurrently.
                    for j in range(jc):
                        nc.tensor.matmul(
                            eA[64:65, :], u_sb[:, j:j + 1],
                            t[:, j, 0:MM_N],
                            start=j == 0, stop=j == jc - 1,
                            tile_position=(0, 64),
                        )
                        nc.tensor.matmul(
                            eB[0:1, :], u_sb[:, j:j + 1],
                            t[:, j, MM_N:2 * MM_N],
                            start=j == 0, stop=j == jc - 1,
                            tile_position=(0, 0),
                        )
                    for ss, eps, gp in ((sA, eA, 64), (sB, eB, 0)):
                        sl = slice(ss * MM_N, (ss + 1) * MM_N)
                        nc.scalar.activation(
                            p_exp[:, sl], eps[gp:gp + 1, :],
                            mybir.ActivationFunctionType.Exp,
                            bias=bias_sb[:],
                        )
                        if b == bl - 1:
                            # Tail batch: ship each slice as soon as its exp
                            # lands. These ride the sync ring -- idle once
                            # the enc stream ends -- so the DMA issues don't
                            # serialize between exps on the ACT queue.
                            nc.sync.dma_start(out_d[b:b + 1, sl],
                                              p_exp[:, sl])
                if b != bl - 1:
                    # Mid-stream outputs ride the idle SWDGE (gpsimd) ring so
                    # their issue slots never sit between exps on the ACT
                    # queue nor behind enc loads on the sync ring.
                    nc.gpsimd.dma_start(out_d[b:b + 1, :], p_exp[:])
    nc.compile()
    return nc


def _prep_inputs(encoder_outputs, attn_W, v):
    encoder_outputs = np.asarray(encoder_outputs, dtype=np.float32)
    attn_W = np.asarray(attn_W, dtype=np.float32)
    v = np.asarray(v, dtype=np.float32)
    h = attn_W.shape[0]
    jc = h // 128
    np_ = S // (2 * MM_N)
    # u = v[0] @ We in float64 (host-side, tiny)
    u = (v[0].astype(np.float64) @ attn_W[:, h:].astype(np.float64)).astype(np.float16)
    u128 = np.ascontiguousarray(u.reshape(jc, 128).T)  # [128, jc]
    in_maps = []
    for c in range(NCORES):
        sl = encoder_outputs[:, c * BL:(c + 1) * BL, :]
        enc_c = sl.transpose(1, 2, 0).astype(np.float16)   # [BL, H, S]
        # [BL, H, S] -> [BL, 128, np, jc, 1024]: partition p holds rows
        # h = j*128+p of one slice pair, j-contiguous 8KB runs per half-DMA.
        enc_c = np.ascontiguousarray(
            enc_c.reshape(BL, jc, 128, np_, 2 * MM_N).transpose(0, 2, 3, 1, 4))
        in_maps.append({"enc": enc_c, "u": u128})
    return in_maps


def run(encoder_outputs, rnn_hidden, attn_W, attn_b, v, trace=False, **bass_kwargs):
    in_maps = _prep_inputs(encoder_outputs, attn_W, v)
    nc = build_nc()
    res = run_bass_kernel_spmd(
        nc, in_maps, list(range(NCORES)), trace=trace, **bass_kwargs
    )
    num = np.concatenate([r["out"] for r in res.results], axis=0)    # [B, S]
    # normalize on host: the constant exp bias cancels in the division
    num = num.astype(np.float64)
    out = num / num.sum(axis=1, keepdims=True)
    return out[:, None, :].astype(np.float32), res


def kernel(encoder_outputs, rnn_hidden, attn_W, attn_b, v):
    out, _ = run(encoder_outputs, rnn_hidden, attn_W, attn_b, v)
    return out
